# revision 69
# baseline (speedup 1.0000x reference)
"""GCN layer (2x gcn_conv with GELU) on 8 Trainium2 NeuronCores.

Contract: kernel(**inputs) takes the FULL inputs of reference.setup_inputs()
and returns the FULL [100000, 64] float32 output.

Design (v3):
- Nodes are relabeled into 100352 virtual slots (8 cores x 98 groups x
  128) by a degree-balancing round-robin deal, so every 128-slot group
  has a near-equal degree sum and the shared SPMD program's
  max-over-cores cell padding is minimal. The output is un-permuted on
  the host.
- Layer 1 dst-sharded: edges become tokens that dma_gather bf16 x rows
  (256B elems) from a replicated x copy; aggregation is G^T@S on TensorE
  into bank-packed PSUM accumulators persistent across a group batch
  (S = one-hot-with-coef built per 128-token column on DVE in bf16, 4x
  mode). Self loops are injected as one xv^T @ diag(dinv^2) matmul per
  group (cheaper than gather tokens, whose window placement differs per
  core and inflates the shared padding). The aggregate comes out
  transposed ([fin, dst]) so the dense transform needs no pre-transpose:
  h1T = W1^T @ mT, GELU with per-partition bias on the Act engine,
  g2T = W2^T @ z1T, one PE transpose back, scale by dinv.
- Layer 2 src-sharded: each core owns the edges whose SOURCE is in its
  shard, gathers its local g2 rows (single 12544-row index window) and
  partial-aggregates over all 784 global dst groups; partials are copied
  bank-at-a-time to bf16 staging on the Act engine, written with merged
  DMAs, and a chunked ReduceScatter(add) (8 group-aligned chunks, small
  tail) returns each core's reduced dst shard. Final: + self g2, * dinv,
  + b2.
- No AllGather anywhere (the v1 bottleneck: 13 chunks x ~64us serialized
  on the collective cores).
"""
import sys
sys.path.insert(0, "/opt/trn_rl_repo")

import numpy as np
import os
import ml_dtypes

BF16 = ml_dtypes.bfloat16

PHASE = int(os.environ.get("GCN_PHASE", "3"))  # 1=L1 only, 3=full

N = 100000
FIN = 128
FOUT = 64
NC = 8
GSZ = 128               # group size
LGP = (-(-N // NC) + GSZ - 1) // GSZ  # 98 local groups (L1 / final)
RS = LGP * GSZ          # 12544 virtual rows per core (group-aligned shard)
NV = NC * RS            # 100352 virtual slots (352 zero-degree phantoms)
GGP = NC * LGP          # 784 global groups (L2 partials)
WIN = 32768             # L1 src index window (int16 limit)
NW = (NV + WIN - 1) // WIN    # 4 windows
B1 = 12                 # L1 groups per PSUM batch
CAP = 8192              # max tokens per dma_gather
NCH = 1                 # ReduceScatter chunks


def _chunk_bounds():
    """Group-aligned per-rank chunk boundaries R[0..NCH]; the last chunk is
    a single group so the final ReduceScatter exposes little tail latency."""
    if NCH == 1:
        return [0, RS]
    if NCH == 3:
        a = int((LGP - 1) * 0.8)
        return [0, a * GSZ, (LGP - 1) * GSZ, RS]
    q, r = divmod(LGP - 1, NCH - 1)
    R = [0]
    for i in range(NCH - 1):
        R.append(R[-1] + (q + (1 if i < r else 0)) * GSZ)
    R.append(RS)
    assert R[NCH - 1] < RS
    return R


def _set_config(**kw):
    g = globals()
    g.update(kw)
    g["LGP"] = (-(-g["N"] // g["NC"]) + g["GSZ"] - 1) // g["GSZ"]
    g["RS"] = g["LGP"] * g["GSZ"]
    g["NV"] = g["NC"] * g["RS"]
    g["GGP"] = g["NC"] * g["LGP"]
    g["NW"] = (g["NV"] + g["WIN"] - 1) // g["WIN"]
    _cache.clear()

_cache = {}


# ----------------------------------------------------------------- host side

def _build_stream(rows_by_core, gl_by_core, dl_by_core, coef_by_core,
                  n_groups, group_order, n_win, batch_sz):
    """Shared SPMD program structure + per-core token data for one layer.

    rows are indices into the gather source; window = rows // WIN when
    n_win > 1 else 0. Tokens are packed into dma_gather calls per
    (batch, window); mdescs [call, col, group, mi, start, stop] carry
    batch-level accumulate start/stop flags.
    """
    has_coef = coef_by_core is not None
    per_core = []
    cell_cnt = np.zeros((NC, n_groups, n_win), dtype=np.int64)
    for c in range(NC):
        rows = rows_by_core[c]
        gl, dl = gl_by_core[c], dl_by_core[c]
        win = (rows // WIN) if n_win > 1 else np.zeros_like(rows)
        order = np.lexsort((rows, win, gl))
        rows, gl, dl, win = rows[order], gl[order], dl[order], win[order]
        coef = coef_by_core[c][order] if has_coef else None
        np.add.at(cell_cnt[c], (gl, win), 1)
        flat_sizes = cell_cnt[c].reshape(-1)
        starts = np.concatenate([[0], np.cumsum(flat_sizes)[:-1]]
                                ).reshape(n_groups, n_win)
        per_core.append(dict(rows=rows, dl=dl.astype(np.float32), coef=coef,
                             starts=starts))

    cell_max = cell_cnt.max(axis=0)
    cell_pad = ((cell_max + 15) // 16) * 16  # idx rows wrap in 16s
    empty = cell_pad.sum(axis=1) == 0
    cell_pad[empty, 0] = 16     # keep every group writable (zero S)

    batches = [group_order[b:b + batch_sz]
               for b in range(0, len(group_order), batch_sz)]
    calls = []
    mdescs = []      # [call_i, col, g, m_index, start, stop]
    total_cols_idx = 0

    def close_call(bi, w, gext, pos):
        nonlocal total_cols_idx
        ntok = ((pos + 127) // 128) * 128
        if ntok == 0:
            return
        ci = len(calls)
        calls.append(dict(batch=bi, w=w, ntok=ntok, gext=dict(gext),
                          idx_col0=total_cols_idx))
        total_cols_idx += ntok // 16
        for j in range(ntok // 128):
            lo, hi = j * 128, (j + 1) * 128
            for g, (s, e) in gext.items():
                if s < hi and e > lo:
                    mdescs.append([ci, j, g, len(mdescs), False, False])

    for bi, groups in enumerate(batches):
        for w in range(n_win):
            gext = {}
            pos = 0
            for g in groups:
                sz = int(cell_pad[g, w])
                if not sz:
                    continue
                assert sz <= CAP, f"cell {g},{w} = {sz} exceeds CAP"
                if pos + sz > CAP:
                    close_call(bi, w, gext, pos)
                    gext, pos = {}, 0
                gext[g] = (pos, pos + sz)
                pos += sz
            close_call(bi, w, gext, pos)

    # batch-level accumulate start/stop
    seen_first = set()
    last_of = {}
    for m in mdescs:
        key = (calls[m[0]]["batch"], m[2])
        if key not in seen_first:
            m[4] = True
            seen_first.add(key)
        last_of[key] = m
    for m in last_of.values():
        m[5] = True
    M = len(mdescs)

    idx_arr = np.zeros((NC, 16, total_cols_idx), dtype=np.int16)
    dst_arr = np.full((NC, M, 128), -1.0, dtype=np.float32)
    coef_arr = np.zeros((NC, M, 128), dtype=np.float32) if has_coef else None

    for c in range(NC):
        pc = per_core[c]
        for call in calls:
            w = call["w"]
            ntok = call["ntok"]
            stream_rows = np.zeros(ntok, dtype=np.int64)
            for g, (s, e) in call["gext"].items():
                cnt = int(cell_cnt[c, g, w])
                st = pc["starts"][g, w]
                if cnt:
                    base = w * WIN if n_win > 1 else 0
                    stream_rows[s:s + cnt] = pc["rows"][st:st + cnt] - base
                    stream_rows[s + cnt:e] = stream_rows[s + cnt - 1]
            c0 = call["idx_col0"]
            idx_arr[c, :, c0:c0 + ntok // 16] = (
                stream_rows.astype(np.int16).reshape(-1, 16).T)
        for ci, j, g, mi, _, _ in mdescs:
            call = calls[ci]
            w = call["w"]
            s, e = call["gext"][g]
            lo, hi = j * 128, (j + 1) * 128
            a = max(s, lo)
            cnt = int(cell_cnt[c, g, w])
            st = pc["starts"][g, w]
            real_hi = min(hi, s + cnt)
            if real_hi > a:
                k0, k1 = a - s, real_hi - s
                dst_arr[c, mi, a - lo:real_hi - lo] = pc["dl"][st + k0:st + k1]
                if has_coef:
                    coef_arr[c, mi, a - lo:real_hi - lo] = \
                        pc["coef"][st + k0:st + k1]

    Mp = ((M + 7) // 8) * 8
    if Mp != M:
        dst_arr = np.concatenate(
            [dst_arr, np.full((NC, Mp - M, 128), -1.0, np.float32)], axis=1)
        if has_coef:
            coef_arr = np.concatenate(
                [coef_arr, np.zeros((NC, Mp - M, 128), np.float32)], axis=1)
    out = dict(calls=calls, mdescs=mdescs, M=Mp,
               idx=np.tile(idx_arr, (1, 8, 1)),
               dst=np.ascontiguousarray(dst_arr.transpose(0, 2, 1)),
               total_idx_cols=total_cols_idx,
               batches=batches,
               max_ntok=max(c_["ntok"] for c_ in calls))
    if has_coef:
        out["coef"] = np.ascontiguousarray(coef_arr.transpose(0, 2, 1))
    return out


import bisect


def _l2_group_pieces(g, R):
    """Split global group g's dst rows into pieces of the chunked partial
    buffers: [(k, dest_row, p0, p1)] — psum partitions [p0,p1) go to
    partial_k[dest_row : dest_row + p1-p0]."""
    d0, d1 = g * GSZ, min((g + 1) * GSZ, NV)
    pieces = []
    d = d0
    while d < d1:
        c2, r = d // RS, d % RS
        k = bisect.bisect_right(R, r) - 1
        ln = min(d1 - d, RS - r, R[k + 1] - r)
        pieces.append((k, c2 * (R[k + 1] - R[k]) + (r - R[k]),
                       d - d0, d - d0 + ln))
        d += ln
    return pieces


def _l2_batch_dmas(groups, R, per_bank=8):
    """Merge a batch's per-group partial pieces into large DMA ops.

    Batch staging = one [128, per_bank*64] f32 tile per bank; slice i of
    the batch holds group groups[i]. Returns ops:
      ('full', k, dest_row, bank, c0, c1)   src stage[b][:, c0:c1, :]
      ('part', k, dest_row, bank, c, p0, p1) src stage[b][p0:p1, c, :]
    """
    ops = []
    cur = None          # [k, dr, bank, c0, c1]
    for i, g in enumerate(groups):
        b, c = i // per_bank, i % per_bank
        for k, dr, p0, p1 in _l2_group_pieces(g, R):
            full = (p0 == 0 and p1 == 128)
            if (full and cur is not None and cur[0] == k and cur[2] == b
                    and cur[4] == c
                    and dr == cur[1] + (cur[4] - cur[3]) * 128):
                cur[4] += 1
            else:
                if cur is not None:
                    ops.append(("full", *cur))
                    cur = None
                if full:
                    cur = [k, dr, b, c, c + 1]
                else:
                    ops.append(("part", k, dr, b, c, p0, p1))
    if cur is not None:
        ops.append(("full", *cur))
    return ops


def _slot_permutation(deg):
    """Relabel nodes into NV virtual slots so every 128-slot group has a
    near-equal degree sum (deal nodes round-robin over the GGP groups in
    descending-degree order; underfull groups keep zero-degree phantom
    slots at their tail). Shards are exactly LGP groups, so the balance
    applies to L1's local groups and L2's global groups alike."""
    order = np.argsort(-deg, kind="stable")
    fill = np.zeros(GGP, dtype=np.int64)
    slot_of = np.empty(N, dtype=np.int64)
    g = 0
    for v in order:
        while fill[g] >= GSZ:
            g = (g + 1) % GGP
        slot_of[v] = g * GSZ + fill[g]
        fill[g] += 1
        g = (g + 1) % GGP
    return slot_of


def _preprocess(x, edge_index, W1, b1, W2, b2):
    src0 = np.asarray(edge_index[0], dtype=np.int64)
    dst0 = np.asarray(edge_index[1], dtype=np.int64)
    deg0 = np.bincount(dst0, minlength=N).astype(np.float32) + 1.0
    slot_of = _slot_permutation(deg0)
    src = slot_of[src0]
    dst = slot_of[dst0]
    xpad = np.zeros((NV, FIN), np.float32)
    xpad[slot_of] = np.asarray(x, np.float32)
    x = xpad
    dinv = np.zeros(NV, np.float32)
    dinv[slot_of] = (1.0 / np.sqrt(deg0)).astype(np.float32)

    # ---- L1: dst-sharded tokens over x (self loops are injected as one
    # xv^T @ diag(dinv^2) matmul per group, not as gather tokens — their
    # window placement differs per core and would inflate the shared
    # cell padding)
    core1 = dst // RS
    l1 = dict(rows=[], gl=[], dl=[], coef=[])
    for c in range(NC):
        m = core1 == c
        s, d = src[m], dst[m]
        rl = d - c * RS
        l1["rows"].append(s)
        l1["gl"].append(rl // GSZ)
        l1["dl"].append(rl % GSZ)
        l1["coef"].append((dinv[s] * dinv[d]).astype(np.float32))
    L1 = _build_stream(l1["rows"], l1["gl"], l1["dl"], l1["coef"],
                       LGP, list(range(LGP)), NW, B1)

    # ---- L2: src-sharded tokens over local g2 rows, global dst groups
    core2 = src // RS
    l2 = dict(rows=[], gl=[], dl=[])
    for c in range(NC):
        m = core2 == c
        s, d = src[m], dst[m]
        l2["rows"].append(s - c * RS)
        l2["gl"].append(d // GSZ)
        l2["dl"].append(d % GSZ)
    # group order: sort by first RS chunk each group touches
    R = _chunk_bounds()
    kmin = np.array([min(k for k, _, _, _ in _l2_group_pieces(g, R))
                     for g in range(GGP)])
    order = list(np.argsort(kmin, kind="stable"))
    B2 = 32
    L2 = _build_stream(l2["rows"], l2["gl"], l2["dl"], None,
                       GGP, order, 1, B2)
    # after which batch can RS chunk k fire? (last batch holding a group
    # with kmin <= k)
    pos_of = {g: i for i, g in enumerate(order)}
    rs_batch = []
    for k in range(NCH):
        last = max(pos_of[g] for g in range(GGP) if kmin[g] <= k)
        rs_batch.append(last // B2)

    dinv_pc = np.zeros((NC, 128, LGP), dtype=np.float32)
    for c in range(NC):
        dinv_pc[c] = dinv[c * RS:(c + 1) * RS].reshape(LGP, GSZ).T
    diag2_pc = np.zeros((NC, 128, LGP, 128), dtype=np.float32)
    idx128 = np.arange(128)
    for c in range(NC):
        diag2_pc[c, idx128, :, idx128] = (dinv_pc[c] ** 2)
    diag2_pc = np.ascontiguousarray(
        diag2_pc.reshape(NC, 128, LGP * 128).astype(BF16))
    xbf_full = np.ascontiguousarray(np.asarray(x, np.float32).astype(BF16))
    xown_pc = np.stack([
        np.ascontiguousarray(
            xbf_full[c * RS:(c + 1) * RS].reshape(LGP, GSZ, FIN)
            .transpose(1, 0, 2).reshape(GSZ, LGP * FIN))
        for c in range(NC)])

    b1col = np.asarray(b1, np.float32).reshape(FOUT, 1)
    b2bc = np.tile(np.asarray(b2, np.float32)[None, :], (128, 1))
    return dict(L1=L1, L2=L2, rs_batch=rs_batch, R=R, dinv_pc=dinv_pc,
                diag2_pc=diag2_pc, xown_pc=xown_pc, slot_of=slot_of,
                xbf=xbf_full,
                b1col=b1col, b2bc=b2bc,
                W1bf=np.ascontiguousarray(np.asarray(W1, np.float32).astype(BF16)),
                W2bf=np.ascontiguousarray(np.asarray(W2, np.float32).astype(BF16)))


# --------------------------------------------------------------- device side

def _build_nc(pp, act="gelu"):
    import concourse.bacc as bacc
    import concourse.tile as tile
    from concourse import mybir
    from concourse.masks import make_identity

    L1, L2 = pp["L1"], pp["L2"]
    nc = bacc.Bacc(num_devices=NC)
    f32 = mybir.dt.float32
    bf16 = mybir.dt.bfloat16
    i16 = mybir.dt.int16

    t_xbf = nc.dram_tensor("xbf", [NV, FIN], bf16, kind="ExternalInput")
    t_idx1 = nc.dram_tensor("idx1", [128, L1["total_idx_cols"]], i16,
                            kind="ExternalInput")
    t_idx2 = nc.dram_tensor("idx2", [128, L2["total_idx_cols"]], i16,
                            kind="ExternalInput")
    t_dst1 = nc.dram_tensor("dst1", [128, L1["M"]], f32, kind="ExternalInput")
    t_coef1 = nc.dram_tensor("coef1", [128, L1["M"]], f32,
                             kind="ExternalInput")
    t_dst2 = nc.dram_tensor("dst2", [128, L2["M"]], bf16,
                            kind="ExternalInput")
    t_dinv = nc.dram_tensor("dinv_pc", [128, LGP], f32, kind="ExternalInput")
    t_diag2 = nc.dram_tensor("diag2", [128, LGP * GSZ], bf16,
                              kind="ExternalInput")
    t_xown = nc.dram_tensor("xown", [GSZ, LGP * FIN], bf16,
                            kind="ExternalInput")
    t_w1 = nc.dram_tensor("W1bf", [FIN, FOUT], bf16, kind="ExternalInput")
    t_w2 = nc.dram_tensor("W2bf", [FOUT, FOUT], bf16, kind="ExternalInput")
    t_b1c = nc.dram_tensor("b1col", [FOUT, 1], f32, kind="ExternalInput")
    t_b2 = nc.dram_tensor("b2bc", [128, FOUT], f32, kind="ExternalInput")
    t_out = nc.dram_tensor("out", [RS, FOUT], f32, kind="ExternalOutput")

    t_g2own = nc.dram_tensor("g2own", [LGP * GSZ, FIN], bf16, kind="Internal")
    R = pp["R"]
    clen = [R[k + 1] - R[k] for k in range(NCH)]
    t_part = [nc.dram_tensor(f"part{k}", [NC * clen[k], FOUT], bf16,
                             kind="Internal") for k in range(NCH)]
    t_z2red = [nc.dram_tensor(f"z2red{k}", [clen[k], FOUT], bf16,
                              kind="Internal") for k in range(NCH)]

    actf = {"gelu": mybir.ActivationFunctionType.Gelu,
            "tanh": mybir.ActivationFunctionType.Tanh}[act]
    winlen = [min(WIN, NV - w * WIN) for w in range(NW)]

    def _bank_flags(L, per_bank):
        """PSUM groups are bank-granular (2KB zero regions): compute, per
        mdesc, whether its matmul is the first/last touching its (batch,
        bank). The bank's first matmul starts (zeroing the whole bank);
        its last one stops."""
        gpos = [{g: i for i, g in enumerate(gs)} for gs in L["batches"]]
        seen, lastm = {}, {}
        for ci, j, g, mi, st, sp in L["mdescs"]:
            bi = L["calls"][ci]["batch"]
            key = (bi, gpos[bi][g] // per_bank)
            if key not in seen:
                seen[key] = mi
            lastm[key] = mi
        return set(seen.values()), set(lastm.values())

    l1_first, l1_last = _bank_flags(L1, 4)
    l2_first, l2_last = _bank_flags(L2, 8)

    with tile.TileContext(nc) as tc:
        with (
            tc.tile_pool(name="const", bufs=1) as cp,
            tc.tile_pool(name="persist", bufs=1) as pers,
        ):
            identb = cp.tile([128, 128], bf16)
            make_identity(nc, identb[:])
            iota_i = cp.tile([128, 128], mybir.dt.int32)
            nc.gpsimd.iota(iota_i[:], pattern=[[1, 128]], base=0,
                           channel_multiplier=0)
            iota_b = cp.tile([128, 128], bf16)
            nc.vector.tensor_copy(iota_b[:], iota_i[:])
            iota8_i = cp.tile([128, 128, 8], mybir.dt.int32)
            nc.gpsimd.iota(iota8_i[:], pattern=[[1, 128], [0, 8]], base=0,
                           channel_multiplier=0)
            iota8_b = cp.tile([128, 128, 8], bf16)
            nc.vector.tensor_copy(iota8_b[:], iota8_i[:])
            w1_t = cp.tile([FIN, FOUT], bf16)
            w2_t = cp.tile([FOUT, FOUT], bf16)
            b1_t = cp.tile([FOUT, 1], f32)
            b2_t = cp.tile([128, FOUT], f32)
            dinv_t = cp.tile([128, LGP], f32)
            for tt_, src_t in ((w1_t, t_w1), (w2_t, t_w2), (b1_t, t_b1c),
                               (b2_t, t_b2), (dinv_t, t_dinv)):
                nc.sync.dma_start(tt_[:], src_t[:, :])
            # big late-needed consts go via the idle Act engine's HWDGE so
            # they don't delay the first idx loads on SP
            diag2_t = cp.tile([128, LGP, GSZ], bf16)
            nc.scalar.dma_start(diag2_t[:],
                                t_diag2[:, :].rearrange("p (g d) -> p g d",
                                                        d=GSZ))
            xown_t = cp.tile([128, LGP, FIN], bf16)
            nc.scalar.dma_start(xown_t[:],
                                t_xown[:, :].rearrange("p (g e) -> p g e",
                                                       e=FIN))
            dst1_t = cp.tile([128, L1["M"]], f32)
            coef1_t = cp.tile([128, L1["M"]], f32)
            dst2_t = cp.tile([128, L2["M"]], bf16)
            nc.sync.dma_start(dst1_t[:], t_dst1[:, :])
            nc.sync.dma_start(coef1_t[:], t_coef1[:, :])
            nc.scalar.dma_start(dst2_t[:], t_dst2[:, :])
            g2own_sb = pers.tile([128, LGP, FIN], bf16)
            nc.gpsimd.memset(g2own_sb[:], 0.0)

            # ---------------- layer 1 (dst-sharded, transposed aggregation)
            def run_l1():
                with (
                    tc.tile_pool(name="gat1", bufs=3) as gp_,
                    tc.tile_pool(name="idx1", bufs=3) as ip_,
                    tc.tile_pool(name="agg1", bufs=6, space="PSUM") as ap_,
                    tc.tile_pool(name="s1", bufs=16) as sp_,
                    tc.tile_pool(name="post1", bufs=4) as wp_,
                    tc.tile_pool(name="pp1", bufs=2, space="PSUM") as pp_,
                ):
                    mi_by_call = {}
                    for m in L1["mdescs"]:
                        mi_by_call.setdefault(m[0], []).append(m)
                    cur_bi = -1
                    banks = []
                    gpos = {}

                    def agg_ap(g):
                        p = gpos[g]
                        return banks[p // 4][:, (p % 4) * 128:
                                             (p % 4) * 128 + 128]

                    for ci, call in enumerate(L1["calls"]):
                        ntok = call["ntok"]
                        ncols = ntok // 128
                        w = call["w"]
                        bi = call["batch"]
                        if bi != cur_bi:
                            cur_bi = bi
                            groups_b = L1["batches"][bi]
                            gpos = {g: i for i, g in enumerate(groups_b)}
                            banks = [ap_.tile([128, 512], f32, tag="a1",
                                              name="agg1p", space="PSUM")
                                     for _ in range((len(groups_b) + 3) // 4)]
                        gtile = gp_.tile([128, L1["max_ntok"] // 128, FIN],
                                         bf16, tag="g1")
                        idxt = ip_.tile([128, L1["max_ntok"] // 16], i16,
                                        tag="i1")
                        c0 = call["idx_col0"]
                        nc.sync.dma_start(idxt[:, :ntok // 16],
                                          t_idx1[:, c0:c0 + ntok // 16])
                        nc.gpsimd.dma_gather(
                            out_ap=gtile[:, :ncols, :],
                            in_ap=t_xbf[w * WIN:w * WIN + winlen[w], :],
                            idxs_ap=idxt[:, :ntok // 16],
                            num_idxs=ntok,
                            num_idxs_reg=ntok,
                            elem_size=FIN,
                            single_packet=True,
                        )
                        for _, j, g, mi, st, sp in mi_by_call.get(ci, []):
                            S = sp_.tile([128, 128], bf16, tag="S1")
                            nc.vector.tensor_scalar(
                                out=S[:], in0=iota_b[:],
                                scalar1=dst1_t[:, mi:mi + 1],
                                scalar2=coef1_t[:, mi:mi + 1],
                                op0=mybir.AluOpType.is_equal,
                                op1=mybir.AluOpType.mult)
                            # mT[g] += gtile_j^T @ S  -> [fin, dst]
                            nc.tensor.matmul(agg_ap(g), lhsT=gtile[:, j, :],
                                             rhs=S[:], start=mi in l1_first,
                                             stop=False)
                        is_last = (ci + 1 == len(L1["calls"])
                                   or L1["calls"][ci + 1]["batch"] != bi)
                        if is_last:
                            gs_b = L1["batches"][bi]
                            for i, g in enumerate(gs_b):
                                # self loop: mT[g] += xv^T @ diag(dinv^2);
                                # the bank's last matmul carries stop
                                nc.tensor.matmul(
                                    agg_ap(g), lhsT=xown_t[:, g, :],
                                    rhs=diag2_t[:, g, :], start=False,
                                    stop=(i % 4 == 3 or i == len(gs_b) - 1))
                            for g in gs_b:
                                post_l1(g, agg_ap(g), wp_, pp_)
                            g0 = gs_b[0]
                            nc.sync.dma_start(
                                t_g2own[g0 * GSZ:(g0 + len(gs_b)) * GSZ, :]
                                .rearrange("(g p) e -> p g e", p=128),
                                g2own_sb[:, g0:g0 + len(gs_b), :])

            def post_l1(g, mT_p, wp_, pp_):
                mT = wp_.tile([128, 128], bf16, tag="mT")
                nc.scalar.activation(mT[:], mT_p,
                                     mybir.ActivationFunctionType.Copy)
                bank = pp_.tile([128, 512], f32, tag="pb", space="PSUM")
                h1T_p = bank[:FOUT, 0:128]
                g2T_p = bank[:FOUT, 128:256]
                g2p = bank[:, 256:288].bitcast(bf16)   # [128, 64] bf16
                nc.tensor.matmul(h1T_p, lhsT=w1_t[:], rhs=mT[:], start=True,
                                 stop=True)
                z1T = wp_.tile([FOUT, 128], bf16, tag="z1T")
                nc.scalar.activation(z1T[:], h1T_p, actf, bias=b1_t[:, 0:1])
                nc.tensor.matmul(g2T_p, lhsT=w2_t[:], rhs=z1T[:], start=True,
                                 stop=True)
                g2T = wp_.tile([FOUT, 128], bf16, tag="g2T")
                nc.scalar.activation(g2T[:], g2T_p,
                                     mybir.ActivationFunctionType.Copy)
                nc.tensor.transpose(g2p, g2T[:], identb[:FOUT, :FOUT])
                nc.vector.tensor_scalar(
                    out=g2own_sb[:, g, 0:FOUT], in0=g2p,
                    scalar1=dinv_t[:, g:g + 1], scalar2=None,
                    op0=mybir.AluOpType.mult)

            run_l1()

            if PHASE == 1:
                # debug: out = g2own rows (f32)
                with tc.tile_pool(name="dbg", bufs=3) as dp_:
                    for g in range(LGP):
                        td = dp_.tile([128, FOUT], f32, tag="td")
                        nc.vector.tensor_copy(td[:], g2own_sb[:, g, 0:FOUT])
                        nrow = min(GSZ, RS - g * GSZ)
                        nc.sync.dma_start(t_out[g * GSZ:g * GSZ + nrow, :],
                                          td[:nrow, :])
                nc.compile()
                return nc

            # ---------------- layer 2 (src-sharded partials + RS)
            rs_of_batch = {}
            for k, bi in enumerate(pp["rs_batch"]):
                rs_of_batch.setdefault(bi, []).append(k)
            # emit chunk-k finals two batches after its RS fires so their
            # RS-dependent waits never head-of-line-block the DVE queue
            n_bat = len(L2["batches"])
            fin_of_batch = {}
            fin_tail = []
            for k, bi in enumerate(pp["rs_batch"]):
                if bi + 4 < n_bat - 1:
                    fin_of_batch.setdefault(bi + 4, []).append(k)
                else:
                    fin_tail.append(k)

            with (
                tc.tile_pool(name="gat2", bufs=4) as gp_,
                tc.tile_pool(name="idx2", bufs=4) as ip_,
                tc.tile_pool(name="agg2", bufs=8, space="PSUM") as ap_,
                tc.tile_pool(name="s2", bufs=4) as sp_,
                tc.tile_pool(name="pw2", bufs=6) as pw_,
                tc.tile_pool(name="fins", bufs=1) as fs_,
                tc.tile_pool(name="fin", bufs=4) as fp_,
            ):
                OB = 7      # groups per out DMA

                def final_chunk(k):
                    # out = dinv*(z2red[k] + g2own) + b2 for the chunk's
                    # local groups; chunk bounds are group-aligned.
                    ncol = (clen[k] + GSZ - 1) // GSZ
                    st_ = fs_.tile([128, ncol, FOUT], bf16, tag=f"fst{k}",
                                   name=f"fst{k}")
                    full = clen[k] // GSZ
                    if full:
                        nc.sync.dma_start(st_[:, :full, :],
                                          t_z2red[k][:full * GSZ, :]
                                          .rearrange("(s p) e -> p s e",
                                                     p=128))
                    if clen[k] % GSZ:
                        nc.sync.dma_start(
                            st_[:clen[k] % GSZ, full, :],
                            t_z2red[k][full * GSZ:clen[k], :])
                    g0k, g1k = R[k] // GSZ, (R[k + 1] + GSZ - 1) // GSZ
                    ot = None
                    for idx, g in enumerate(range(g0k, g1k)):
                        nrow = min(GSZ, RS - g * GSZ)
                        o = idx % OB
                        if o == 0:
                            ot = fp_.tile([128, OB, FOUT], f32, tag="t3")
                        t1 = fp_.tile([128, FOUT], f32, tag="t1")
                        nc.vector.tensor_tensor(
                            out=t1[:nrow, :], in0=st_[:nrow, idx, :],
                            in1=g2own_sb[:nrow, g, 0:FOUT],
                            op=mybir.AluOpType.add)
                        nc.vector.tensor_scalar(
                            out=t1[:nrow, :], in0=t1[:nrow, :],
                            scalar1=dinv_t[:nrow, g:g + 1], scalar2=None,
                            op0=mybir.AluOpType.mult)
                        nc.vector.tensor_tensor(
                            out=ot[:nrow, o, :], in0=t1[:nrow, :],
                            in1=b2_t[:nrow, :], op=mybir.AluOpType.add)
                        if o == OB - 1 or g == g1k - 1:
                            g0 = g - o
                            if nrow == GSZ:
                                nc.sync.dma_start(
                                    t_out[g0 * GSZ:(g + 1) * GSZ, :]
                                    .rearrange("(g p) e -> p g e", p=128),
                                    ot[:, :o + 1, :])
                            else:
                                if o:
                                    nc.sync.dma_start(
                                        t_out[g0 * GSZ:g * GSZ, :]
                                        .rearrange("(g p) e -> p g e",
                                                   p=128),
                                        ot[:, :o, :])
                                nc.sync.dma_start(
                                    t_out[g * GSZ:g * GSZ + nrow, :],
                                    ot[:nrow, o, :])

                mi_by_call = {}
                for m in L2["mdescs"]:
                    mi_by_call.setdefault(m[0], []).append(m)
                cur_bi = -1
                banks = []
                gpos = {}

                def agg_ap2(g):
                    p = gpos[g]
                    return banks[p // 8][:, (p % 8) * FOUT:
                                         (p % 8) * FOUT + FOUT]

                for ci, call in enumerate(L2["calls"]):
                    ntok = call["ntok"]
                    ncols = ntok // 128
                    bi = call["batch"]
                    if bi != cur_bi:
                        cur_bi = bi
                        groups_b = L2["batches"][bi]
                        gpos = {g: i for i, g in enumerate(groups_b)}
                        banks = [ap_.tile([128, 512], f32, tag="a2",
                                          name="agg2p", space="PSUM")
                                 for _ in range((len(groups_b) + 7) // 8)]
                    gtile = gp_.tile([128, L2["max_ntok"] // 128, FIN],
                                     bf16, tag="g2")
                    idxt = ip_.tile([128, L2["max_ntok"] // 16], i16,
                                    tag="i2")
                    c0 = call["idx_col0"]
                    nc.sync.dma_start(idxt[:, :ntok // 16],
                                      t_idx2[:, c0:c0 + ntok // 16])
                    nc.gpsimd.dma_gather(
                        out_ap=gtile[:, :ncols, :],
                        in_ap=t_g2own[:, :],
                        idxs_ap=idxt[:, :ntok // 16],
                        num_idxs=ntok,
                        num_idxs_reg=ntok,
                        elem_size=FIN,
                        single_packet=True,
                    )
                    for _, j, g, mi, st, sp in mi_by_call.get(ci, []):
                        if mi % 8 == 0:
                            # one DVE op builds S for 8 columns:
                            # s8[p, d, q] = (dst2[p, mi+q] == d)
                            s8 = sp_.tile([128, 128, 8], bf16, tag="S2")
                            nc.vector.tensor_tensor(
                                out=s8[:], in0=iota8_b[:],
                                in1=dst2_t[:, mi:mi + 8].unsqueeze(1)
                                .broadcast_to([128, 128, 8]),
                                op=mybir.AluOpType.is_equal)
                        nc.tensor.matmul(agg_ap2(g), lhsT=s8[:, :, mi % 8],
                                         rhs=gtile[:, j, 0:FOUT],
                                         start=mi in l2_first,
                                         stop=mi in l2_last)
                    is_last = (ci + 1 == len(L2["calls"])
                               or L2["calls"][ci + 1]["batch"] != bi)
                    if is_last:
                        gs_b = L2["batches"][bi]
                        stages = []
                        for b in range(len(banks)):
                            nct = min(8, len(gs_b) - b * 8)
                            stg = pw_.tile([128, 8, FOUT], bf16, tag="ps")
                            nc.scalar.activation(
                                stg[:, :nct, :],
                                banks[b][:, :nct * FOUT].rearrange(
                                    "p (c e) -> p c e", e=FOUT),
                                mybir.ActivationFunctionType.Copy)
                            stages.append(stg)
                        for op in _l2_batch_dmas(gs_b, R):
                            if op[0] == "full":
                                _, k, dr, b, c0, c1 = op
                                nc.sync.dma_start(
                                    t_part[k][dr:dr + (c1 - c0) * 128, :]
                                    .rearrange("(c p) e -> p c e", p=128),
                                    stages[b][:, c0:c1, :])
                            else:
                                _, k, dr, b, c, p0, p1 = op
                                nc.sync.dma_start(
                                    t_part[k][dr:dr + (p1 - p0), :],
                                    stages[b][p0:p1, c, :])
                        for k in rs_of_batch.get(bi, []):
                            nc.gpsimd.collective_compute(
                                "ReduceScatter", mybir.AluOpType.add,
                                replica_groups=[list(range(NC))],
                                ins=[t_part[k][:, :]],
                                outs=[t_z2red[k][:, :]])
                        for k in fin_of_batch.get(bi, []):
                            final_chunk(k)
                for k in fin_tail:
                    final_chunk(k)

    nc.compile()
    return nc


def _in_maps(pp):
    maps = []
    for c in range(NC):
        maps.append({
            "xbf": pp["xbf"],
            "idx1": pp["L1"]["idx"][c],
            "idx2": pp["L2"]["idx"][c],
            "dst1": pp["L1"]["dst"][c],
            "coef1": pp["L1"]["coef"][c],
            "dst2": pp["L2"]["dst"][c].astype(BF16),
            "dinv_pc": pp["dinv_pc"][c],
            "diag2": pp["diag2_pc"][c],
            "xown": pp["xown_pc"][c],
            "W1bf": pp["W1bf"], "W2bf": pp["W2bf"],
            "b1col": pp["b1col"], "b2bc": pp["b2bc"],
        })
    return maps


def _run(inputs, act="gelu", trace=False, use_sim=False, trace_kwargs=None):
    x = np.ascontiguousarray(np.asarray(inputs["x"], np.float32))
    key = (hash(np.asarray(inputs["edge_index"]).tobytes()), act, PHASE)
    if key not in _cache:
        pp = _preprocess(x, np.asarray(inputs["edge_index"]),
                         inputs["W1"], inputs["b1"], inputs["W2"],
                         inputs["b2"])
        nc = _build_nc(pp, act=act)
        _cache.clear()
        _cache[key] = (pp, nc)
    pp, nc = _cache[key]

    in_maps = _in_maps(pp)
    if use_sim:
        from concourse.bass_interp import MultiCoreSim
        sim = MultiCoreSim(nc, num_cores=NC)
        for ci, core in sim.cores.items():
            for k, v in in_maps[ci].items():
                core.tensor(k)[:] = v
        sim.simulate()
        outs = [np.array(core.tensor("out"))
                for _, core in sorted(sim.cores.items())]
        return np.concatenate(outs, 0)[pp["slot_of"]], None
    from concourse.bass_utils import run_bass_kernel_spmd
    res = run_bass_kernel_spmd(nc, in_maps, core_ids=list(range(NC)),
                               trace=trace, **(trace_kwargs or {}))
    out = np.concatenate([res.results[c]["out"] for c in range(NC)], 0)
    return out[pp["slot_of"]], res


def kernel(**inputs) -> np.ndarray:
    out, _ = _run(inputs)
    return out


def bench(inputs, act="gelu", iters=8):
    """Measure per-execution device time by chaining `iters` executions of
    the NEFF inside one jit and comparing against a 1-iteration call."""
    import time
    import jax
    from jax.sharding import Mesh, PartitionSpec
    from jax.experimental.shard_map import shard_map
    from concourse import bass2jax as b2j

    key = (hash(np.asarray(inputs["edge_index"]).tobytes()), act, PHASE)
    if key not in _cache:
        _run(inputs, act=act)   # build + correctness path
    pp, nc = _cache[key]
    b2j.install_neuronx_cc_hook()

    in_maps = _in_maps(pp)

    in_names, out_names, out_avals, zero_outs = [], [], [], []
    import concourse.mybir as mb
    pid_name = (nc.partition_id_tensor.name
                if nc.partition_id_tensor is not None else None)
    for alloc in nc.m.functions[0].allocations:
        if not isinstance(alloc, mb.MemoryLocationSet):
            continue
        name = alloc.memorylocations[0].name
        if alloc.kind == "ExternalInput":
            if name == pid_name:
                continue
            in_names.append(name)
        elif alloc.kind == "ExternalOutput":
            out_names.append(name)
            shape = tuple(alloc.tensor_shape)
            dtype = mb.dt.np(alloc.dtype)
            out_avals.append(jax.core.ShapedArray(shape, dtype))
            zero_outs.append(np.zeros(shape, dtype))
    n_params = len(in_names)
    all_names = in_names + out_names
    if pid_name is not None:
        all_names = all_names + [pid_name]

    def one_call(params, outs_in):
        extra = ([b2j.partition_id_tensor()] if pid_name is not None else [])
        outs = b2j._bass_exec_p.bind(
            *params, *outs_in, *extra,
            out_avals=tuple(out_avals),
            in_names=tuple(all_names),
            out_names=tuple(out_names),
            lowering_input_output_aliases=(),
            sim_require_finite=True,
            sim_require_nnan=True,
            nc=nc,
        )
        return list(outs)

    def _body(*args):
        params = list(args[:n_params])
        outs = list(args[n_params:])
        outs = one_call(params, outs)
        return tuple(outs)

    devices = jax.devices()[:NC]
    mesh = Mesh(np.asarray(devices), ("core",))
    specs = (PartitionSpec("core"),)
    per_core = [[np.asarray(m[nm]) for nm in in_names] for m in in_maps]
    concat_in = [np.concatenate([per_core[c][i] for c in range(NC)], 0)
                 for i in range(n_params)]
    concat_zeros = [np.zeros((NC * z.shape[0], *z.shape[1:]), z.dtype)
                    for z in zero_outs]

    nin = n_params + len(out_names)
    fn = jax.jit(shard_map(_body, mesh=mesh,
                           in_specs=specs * nin,
                           out_specs=specs * len(out_names),
                           check_rep=False),
                 donate_argnums=tuple(range(n_params, nin)))
    from jax.sharding import NamedSharding
    shard = NamedSharding(mesh, PartitionSpec("core"))
    dev_in = [jax.device_put(a, shard) for a in concat_in]
    outs = [jax.device_put(a, shard) for a in concat_zeros]
    outs = fn(*dev_in, *outs)          # warm: compile + first exec
    jax.block_until_ready(outs)

    results = {}
    for k in (1, iters):
        best = None
        for _ in range(3):
            t0 = time.perf_counter()
            o = outs
            for _ in range(k):
                o = fn(*dev_in, *o)
            jax.block_until_ready(o)
            dt = time.perf_counter() - t0
            outs = o
            best = dt if best is None else min(best, dt)
        results[k] = best
    per_iter_ns = (results[iters] - results[1]) / (iters - 1) * 1e9
    return per_iter_ns, results


# revision 70
# speedup vs baseline: 2.4812x; 2.4812x over previous
"""GCN layer (2x gcn_conv with GELU) on 8 Trainium2 NeuronCores.

Contract: kernel(**inputs) takes the FULL inputs of reference.setup_inputs()
and returns the FULL [100000, 64] float32 output.

Design (v3):
- Nodes are relabeled into 100352 virtual slots (8 cores x 98 groups x
  128) by a degree-balancing round-robin deal, so every 128-slot group
  has a near-equal degree sum and the shared SPMD program's
  max-over-cores cell padding is minimal. The output is un-permuted on
  the host.
- Layer 1 dst-sharded: edges become tokens that dma_gather bf16 x rows
  (256B elems) from a replicated x copy; aggregation is G^T@S on TensorE
  into bank-packed PSUM accumulators persistent across a group batch
  (S = one-hot-with-coef built per 128-token column on DVE in bf16, 4x
  mode). Self loops are injected as one xv^T @ diag(dinv^2) matmul per
  group (cheaper than gather tokens, whose window placement differs per
  core and inflates the shared padding). The aggregate comes out
  transposed ([fin, dst]) so the dense transform needs no pre-transpose:
  h1T = W1^T @ mT, GELU with per-partition bias on the Act engine,
  g2T = W2^T @ z1T, one PE transpose back, scale by dinv.
- Layer 2 src-sharded: each core owns the edges whose SOURCE is in its
  shard, gathers its local g2 rows (single 12544-row index window) and
  partial-aggregates over all 784 global dst groups; partials are copied
  bank-at-a-time to bf16 staging on the Act engine, written with merged
  DMAs, and a chunked ReduceScatter(add) (8 group-aligned chunks, small
  tail) returns each core's reduced dst shard. Final: + self g2, * dinv,
  + b2.
- No AllGather anywhere (the v1 bottleneck: 13 chunks x ~64us serialized
  on the collective cores).
"""
import sys
sys.path.insert(0, "/opt/trn_rl_repo")

import numpy as np
import os
import ml_dtypes

BF16 = ml_dtypes.bfloat16

PHASE = int(os.environ.get("GCN_PHASE", "3"))  # 1=L1 only, 3=full

N = 100000
FIN = 128
FOUT = 64
NC = 8
GSZ = 128               # group size
LGP = (-(-N // NC) + GSZ - 1) // GSZ  # 98 local groups (L1 / final)
RS = LGP * GSZ          # 12544 virtual rows per core (group-aligned shard)
NV = NC * RS            # 100352 virtual slots (352 zero-degree phantoms)
GGP = NC * LGP          # 784 global groups (L2 partials)
WIN = 32768             # L1 src index window (int16 limit)
NW = (NV + WIN - 1) // WIN    # 4 windows
B1 = 12                 # L1 groups per PSUM batch
CAP = 8192              # max tokens per dma_gather
NCH = 1                 # ReduceScatter chunks


def _chunk_bounds():
    """Group-aligned per-rank chunk boundaries R[0..NCH]; the last chunk is
    a single group so the final ReduceScatter exposes little tail latency."""
    if NCH == 1:
        return [0, RS]
    if NCH == 3:
        a = int((LGP - 1) * 0.8)
        return [0, a * GSZ, (LGP - 1) * GSZ, RS]
    q, r = divmod(LGP - 1, NCH - 1)
    R = [0]
    for i in range(NCH - 1):
        R.append(R[-1] + (q + (1 if i < r else 0)) * GSZ)
    R.append(RS)
    assert R[NCH - 1] < RS
    return R


def _set_config(**kw):
    g = globals()
    g.update(kw)
    g["LGP"] = (-(-g["N"] // g["NC"]) + g["GSZ"] - 1) // g["GSZ"]
    g["RS"] = g["LGP"] * g["GSZ"]
    g["NV"] = g["NC"] * g["RS"]
    g["GGP"] = g["NC"] * g["LGP"]
    g["NW"] = (g["NV"] + g["WIN"] - 1) // g["WIN"]
    _cache.clear()

_cache = {}


# ----------------------------------------------------------------- host side

def _build_stream(rows_by_core, gl_by_core, dl_by_core, coef_by_core,
                  n_groups, group_order, n_win, batch_sz):
    """Shared SPMD program structure + per-core token data for one layer.

    rows are indices into the gather source; window = rows // WIN when
    n_win > 1 else 0. Tokens are packed into dma_gather calls per
    (batch, window); mdescs [call, col, group, mi, start, stop] carry
    batch-level accumulate start/stop flags.
    """
    has_coef = coef_by_core is not None
    per_core = []
    cell_cnt = np.zeros((NC, n_groups, n_win), dtype=np.int64)
    for c in range(NC):
        rows = rows_by_core[c]
        gl, dl = gl_by_core[c], dl_by_core[c]
        win = (rows // WIN) if n_win > 1 else np.zeros_like(rows)
        order = np.lexsort((rows, win, gl))
        rows, gl, dl, win = rows[order], gl[order], dl[order], win[order]
        coef = coef_by_core[c][order] if has_coef else None
        np.add.at(cell_cnt[c], (gl, win), 1)
        flat_sizes = cell_cnt[c].reshape(-1)
        starts = np.concatenate([[0], np.cumsum(flat_sizes)[:-1]]
                                ).reshape(n_groups, n_win)
        per_core.append(dict(rows=rows, dl=dl.astype(np.float32), coef=coef,
                             starts=starts))

    cell_max = cell_cnt.max(axis=0)
    cell_pad = ((cell_max + 15) // 16) * 16  # idx rows wrap in 16s
    empty = cell_pad.sum(axis=1) == 0
    cell_pad[empty, 0] = 16     # keep every group writable (zero S)

    batches = [group_order[b:b + batch_sz]
               for b in range(0, len(group_order), batch_sz)]
    calls = []
    mdescs = []      # [call_i, col, g, m_index, start, stop]
    total_cols_idx = 0

    def close_call(bi, w, gext, pos):
        nonlocal total_cols_idx
        ntok = ((pos + 127) // 128) * 128
        if ntok == 0:
            return
        ci = len(calls)
        calls.append(dict(batch=bi, w=w, ntok=ntok, gext=dict(gext),
                          idx_col0=total_cols_idx))
        total_cols_idx += ntok // 16
        for j in range(ntok // 128):
            lo, hi = j * 128, (j + 1) * 128
            for g, (s, e) in gext.items():
                if s < hi and e > lo:
                    mdescs.append([ci, j, g, len(mdescs), False, False])

    for bi, groups in enumerate(batches):
        for w in range(n_win):
            gext = {}
            pos = 0
            for g in groups:
                sz = int(cell_pad[g, w])
                if not sz:
                    continue
                assert sz <= CAP, f"cell {g},{w} = {sz} exceeds CAP"
                if pos + sz > CAP:
                    close_call(bi, w, gext, pos)
                    gext, pos = {}, 0
                gext[g] = (pos, pos + sz)
                pos += sz
            close_call(bi, w, gext, pos)

    # batch-level accumulate start/stop
    seen_first = set()
    last_of = {}
    for m in mdescs:
        key = (calls[m[0]]["batch"], m[2])
        if key not in seen_first:
            m[4] = True
            seen_first.add(key)
        last_of[key] = m
    for m in last_of.values():
        m[5] = True
    M = len(mdescs)

    idx_arr = np.zeros((NC, 16, total_cols_idx), dtype=np.int16)
    dst_arr = np.full((NC, M, 128), -1.0, dtype=np.float32)
    coef_arr = np.zeros((NC, M, 128), dtype=np.float32) if has_coef else None

    for c in range(NC):
        pc = per_core[c]
        for call in calls:
            w = call["w"]
            ntok = call["ntok"]
            stream_rows = np.zeros(ntok, dtype=np.int64)
            for g, (s, e) in call["gext"].items():
                cnt = int(cell_cnt[c, g, w])
                st = pc["starts"][g, w]
                if cnt:
                    base = w * WIN if n_win > 1 else 0
                    stream_rows[s:s + cnt] = pc["rows"][st:st + cnt] - base
                    stream_rows[s + cnt:e] = stream_rows[s + cnt - 1]
            c0 = call["idx_col0"]
            idx_arr[c, :, c0:c0 + ntok // 16] = (
                stream_rows.astype(np.int16).reshape(-1, 16).T)
        for ci, j, g, mi, _, _ in mdescs:
            call = calls[ci]
            w = call["w"]
            s, e = call["gext"][g]
            lo, hi = j * 128, (j + 1) * 128
            a = max(s, lo)
            cnt = int(cell_cnt[c, g, w])
            st = pc["starts"][g, w]
            real_hi = min(hi, s + cnt)
            if real_hi > a:
                k0, k1 = a - s, real_hi - s
                dst_arr[c, mi, a - lo:real_hi - lo] = pc["dl"][st + k0:st + k1]
                if has_coef:
                    coef_arr[c, mi, a - lo:real_hi - lo] = \
                        pc["coef"][st + k0:st + k1]

    Mp = ((M + 7) // 8) * 8
    if Mp != M:
        dst_arr = np.concatenate(
            [dst_arr, np.full((NC, Mp - M, 128), -1.0, np.float32)], axis=1)
        if has_coef:
            coef_arr = np.concatenate(
                [coef_arr, np.zeros((NC, Mp - M, 128), np.float32)], axis=1)
    out = dict(calls=calls, mdescs=mdescs, M=Mp,
               idx=np.tile(idx_arr, (1, 8, 1)),
               dst=np.ascontiguousarray(dst_arr.transpose(0, 2, 1)),
               total_idx_cols=total_cols_idx,
               batches=batches,
               max_ntok=max(c_["ntok"] for c_ in calls))
    if has_coef:
        out["coef"] = np.ascontiguousarray(coef_arr.transpose(0, 2, 1))
    return out


import bisect


def _l2_group_pieces(g, R):
    """Split global group g's dst rows into pieces of the chunked partial
    buffers: [(k, dest_row, p0, p1)] — psum partitions [p0,p1) go to
    partial_k[dest_row : dest_row + p1-p0]."""
    d0, d1 = g * GSZ, min((g + 1) * GSZ, NV)
    pieces = []
    d = d0
    while d < d1:
        c2, r = d // RS, d % RS
        k = bisect.bisect_right(R, r) - 1
        ln = min(d1 - d, RS - r, R[k + 1] - r)
        pieces.append((k, c2 * (R[k + 1] - R[k]) + (r - R[k]),
                       d - d0, d - d0 + ln))
        d += ln
    return pieces


def _l2_batch_dmas(groups, R, per_bank=8):
    """Merge a batch's per-group partial pieces into large DMA ops.

    Batch staging = one [128, per_bank*64] f32 tile per bank; slice i of
    the batch holds group groups[i]. Returns ops:
      ('full', k, dest_row, bank, c0, c1)   src stage[b][:, c0:c1, :]
      ('part', k, dest_row, bank, c, p0, p1) src stage[b][p0:p1, c, :]
    """
    ops = []
    cur = None          # [k, dr, bank, c0, c1]
    for i, g in enumerate(groups):
        b, c = i // per_bank, i % per_bank
        for k, dr, p0, p1 in _l2_group_pieces(g, R):
            full = (p0 == 0 and p1 == 128)
            if (full and cur is not None and cur[0] == k and cur[2] == b
                    and cur[4] == c
                    and dr == cur[1] + (cur[4] - cur[3]) * 128):
                cur[4] += 1
            else:
                if cur is not None:
                    ops.append(("full", *cur))
                    cur = None
                if full:
                    cur = [k, dr, b, c, c + 1]
                else:
                    ops.append(("part", k, dr, b, c, p0, p1))
    if cur is not None:
        ops.append(("full", *cur))
    return ops


def _slot_permutation(deg):
    """Relabel nodes into NV virtual slots so every 128-slot group has a
    near-equal degree sum (deal nodes round-robin over the GGP groups in
    descending-degree order; underfull groups keep zero-degree phantom
    slots at their tail). Shards are exactly LGP groups, so the balance
    applies to L1's local groups and L2's global groups alike."""
    order = np.argsort(-deg, kind="stable")
    fill = np.zeros(GGP, dtype=np.int64)
    slot_of = np.empty(N, dtype=np.int64)
    g = 0
    for v in order:
        while fill[g] >= GSZ:
            g = (g + 1) % GGP
        slot_of[v] = g * GSZ + fill[g]
        fill[g] += 1
        g = (g + 1) % GGP
    return slot_of


def _preprocess(x, edge_index, W1, b1, W2, b2):
    src0 = np.asarray(edge_index[0], dtype=np.int64)
    dst0 = np.asarray(edge_index[1], dtype=np.int64)
    deg0 = np.bincount(dst0, minlength=N).astype(np.float32) + 1.0
    slot_of = _slot_permutation(deg0)
    src = slot_of[src0]
    dst = slot_of[dst0]
    xpad = np.zeros((NV, FIN), np.float32)
    xpad[slot_of] = np.asarray(x, np.float32)
    x = xpad
    dinv = np.zeros(NV, np.float32)
    dinv[slot_of] = (1.0 / np.sqrt(deg0)).astype(np.float32)

    # ---- L1: dst-sharded tokens over x (self loops are injected as one
    # xv^T @ diag(dinv^2) matmul per group, not as gather tokens — their
    # window placement differs per core and would inflate the shared
    # cell padding)
    core1 = dst // RS
    l1 = dict(rows=[], gl=[], dl=[], coef=[])
    for c in range(NC):
        m = core1 == c
        s, d = src[m], dst[m]
        rl = d - c * RS
        l1["rows"].append(s)
        l1["gl"].append(rl // GSZ)
        l1["dl"].append(rl % GSZ)
        l1["coef"].append((dinv[s] * dinv[d]).astype(np.float32))
    L1 = _build_stream(l1["rows"], l1["gl"], l1["dl"], l1["coef"],
                       LGP, list(range(LGP)), NW, B1)

    # ---- L2: src-sharded tokens over local g2 rows, global dst groups
    core2 = src // RS
    l2 = dict(rows=[], gl=[], dl=[])
    for c in range(NC):
        m = core2 == c
        s, d = src[m], dst[m]
        l2["rows"].append(s - c * RS)
        l2["gl"].append(d // GSZ)
        l2["dl"].append(d % GSZ)
    # group order: sort by first RS chunk each group touches
    R = _chunk_bounds()
    kmin = np.array([min(k for k, _, _, _ in _l2_group_pieces(g, R))
                     for g in range(GGP)])
    order = list(np.argsort(kmin, kind="stable"))
    B2 = 32
    L2 = _build_stream(l2["rows"], l2["gl"], l2["dl"], None,
                       GGP, order, 1, B2)
    # after which batch can RS chunk k fire? (last batch holding a group
    # with kmin <= k)
    pos_of = {g: i for i, g in enumerate(order)}
    rs_batch = []
    for k in range(NCH):
        last = max(pos_of[g] for g in range(GGP) if kmin[g] <= k)
        rs_batch.append(last // B2)

    dinv_pc = np.zeros((NC, 128, LGP), dtype=np.float32)
    for c in range(NC):
        dinv_pc[c] = dinv[c * RS:(c + 1) * RS].reshape(LGP, GSZ).T
    diag2_pc = np.zeros((NC, 128, LGP, 128), dtype=np.float32)
    idx128 = np.arange(128)
    for c in range(NC):
        diag2_pc[c, idx128, :, idx128] = (dinv_pc[c] ** 2)
    diag2_pc = np.ascontiguousarray(
        diag2_pc.reshape(NC, 128, LGP * 128).astype(BF16))
    xbf_full = np.ascontiguousarray(np.asarray(x, np.float32).astype(BF16))
    xown_pc = np.stack([
        np.ascontiguousarray(
            xbf_full[c * RS:(c + 1) * RS].reshape(LGP, GSZ, FIN)
            .transpose(1, 0, 2).reshape(GSZ, LGP * FIN))
        for c in range(NC)])

    b1col = np.asarray(b1, np.float32).reshape(FOUT, 1)
    b2bc = np.tile(np.asarray(b2, np.float32)[None, :], (128, 1))
    return dict(L1=L1, L2=L2, rs_batch=rs_batch, R=R, dinv_pc=dinv_pc,
                diag2_pc=diag2_pc, xown_pc=xown_pc, slot_of=slot_of,
                xbf=xbf_full,
                b1col=b1col, b2bc=b2bc,
                W1bf=np.ascontiguousarray(np.asarray(W1, np.float32).astype(BF16)),
                W2bf=np.ascontiguousarray(np.asarray(W2, np.float32).astype(BF16)))


# --------------------------------------------------------------- device side

def _build_nc(pp, act="gelu"):
    import concourse.bacc as bacc
    import concourse.tile as tile
    from concourse import mybir
    from concourse.masks import make_identity

    L1, L2 = pp["L1"], pp["L2"]
    nc = bacc.Bacc(num_devices=NC)
    f32 = mybir.dt.float32
    bf16 = mybir.dt.bfloat16
    i16 = mybir.dt.int16

    t_xbf = nc.dram_tensor("xbf", [NV, FIN], bf16, kind="ExternalInput")
    t_idx1 = nc.dram_tensor("idx1", [128, L1["total_idx_cols"]], i16,
                            kind="ExternalInput")
    t_idx2 = nc.dram_tensor("idx2", [128, L2["total_idx_cols"]], i16,
                            kind="ExternalInput")
    t_dst1 = nc.dram_tensor("dst1", [128, L1["M"]], f32, kind="ExternalInput")
    t_coef1 = nc.dram_tensor("coef1", [128, L1["M"]], f32,
                             kind="ExternalInput")
    t_dst2 = nc.dram_tensor("dst2", [128, L2["M"]], bf16,
                            kind="ExternalInput")
    t_dinv = nc.dram_tensor("dinv_pc", [128, LGP], f32, kind="ExternalInput")
    t_diag2 = nc.dram_tensor("diag2", [128, LGP * GSZ], bf16,
                              kind="ExternalInput")
    t_xown = nc.dram_tensor("xown", [GSZ, LGP * FIN], bf16,
                            kind="ExternalInput")
    t_w1 = nc.dram_tensor("W1bf", [FIN, FOUT], bf16, kind="ExternalInput")
    t_w2 = nc.dram_tensor("W2bf", [FOUT, FOUT], bf16, kind="ExternalInput")
    t_b1c = nc.dram_tensor("b1col", [FOUT, 1], f32, kind="ExternalInput")
    t_b2 = nc.dram_tensor("b2bc", [128, FOUT], f32, kind="ExternalInput")
    t_out = nc.dram_tensor("out", [RS, FOUT], f32, kind="ExternalOutput")

    t_g2own = nc.dram_tensor("g2own", [LGP * GSZ, FIN], bf16, kind="Internal")
    R = pp["R"]
    clen = [R[k + 1] - R[k] for k in range(NCH)]
    t_part = [nc.dram_tensor(f"part{k}", [NC * clen[k], FOUT], bf16,
                             kind="Internal") for k in range(NCH)]
    t_z2red = [nc.dram_tensor(f"z2red{k}", [clen[k], FOUT], bf16,
                              kind="Internal") for k in range(NCH)]

    actf = {"gelu": mybir.ActivationFunctionType.Gelu,
            "tanh": mybir.ActivationFunctionType.Tanh}[act]
    winlen = [min(WIN, NV - w * WIN) for w in range(NW)]

    def _bank_flags(L, per_bank):
        """PSUM groups are bank-granular (2KB zero regions): compute, per
        mdesc, whether its matmul is the first/last touching its (batch,
        bank). The bank's first matmul starts (zeroing the whole bank);
        its last one stops."""
        gpos = [{g: i for i, g in enumerate(gs)} for gs in L["batches"]]
        seen, lastm = {}, {}
        for ci, j, g, mi, st, sp in L["mdescs"]:
            bi = L["calls"][ci]["batch"]
            key = (bi, gpos[bi][g] // per_bank)
            if key not in seen:
                seen[key] = mi
            lastm[key] = mi
        return set(seen.values()), set(lastm.values())

    l1_first, l1_last = _bank_flags(L1, 4)
    l2_first, l2_last = _bank_flags(L2, 8)

    with tile.TileContext(nc) as tc:
        with (
            tc.tile_pool(name="const", bufs=1) as cp,
            tc.tile_pool(name="persist", bufs=1) as pers,
        ):
            identb = cp.tile([128, 128], bf16)
            make_identity(nc, identb[:])
            iota_i = cp.tile([128, 128], mybir.dt.int32)
            nc.gpsimd.iota(iota_i[:], pattern=[[1, 128]], base=0,
                           channel_multiplier=0)
            iota_b = cp.tile([128, 128], bf16)
            nc.vector.tensor_copy(iota_b[:], iota_i[:])
            iota8_i = cp.tile([128, 128, 8], mybir.dt.int32)
            nc.gpsimd.iota(iota8_i[:], pattern=[[1, 128], [0, 8]], base=0,
                           channel_multiplier=0)
            iota8_b = cp.tile([128, 128, 8], bf16)
            nc.vector.tensor_copy(iota8_b[:], iota8_i[:])
            w1_t = cp.tile([FIN, FOUT], bf16)
            w2_t = cp.tile([FOUT, FOUT], bf16)
            b1_t = cp.tile([FOUT, 1], f32)
            b2_t = cp.tile([128, FOUT], f32)
            dinv_t = cp.tile([128, LGP], f32)
            for tt_, src_t in ((w1_t, t_w1), (w2_t, t_w2), (b1_t, t_b1c),
                               (b2_t, t_b2), (dinv_t, t_dinv)):
                nc.sync.dma_start(tt_[:], src_t[:, :])
            # big late-needed consts go via the idle Act engine's HWDGE so
            # they don't delay the first idx loads on SP
            diag2_t = cp.tile([128, LGP, GSZ], bf16)
            nc.scalar.dma_start(diag2_t[:],
                                t_diag2[:, :].rearrange("p (g d) -> p g d",
                                                        d=GSZ))
            xown_t = cp.tile([128, LGP, FIN], bf16)
            nc.scalar.dma_start(xown_t[:],
                                t_xown[:, :].rearrange("p (g e) -> p g e",
                                                       e=FIN))
            dst1_t = cp.tile([128, L1["M"]], f32)
            coef1_t = cp.tile([128, L1["M"]], f32)
            dst2_t = cp.tile([128, L2["M"]], bf16)
            nc.sync.dma_start(dst1_t[:], t_dst1[:, :])
            nc.sync.dma_start(coef1_t[:], t_coef1[:, :])
            nc.scalar.dma_start(dst2_t[:], t_dst2[:, :])
            g2own_sb = pers.tile([128, LGP, FIN], bf16)
            nc.gpsimd.memset(g2own_sb[:], 0.0)

            # ---------------- layer 1 (dst-sharded, transposed aggregation)
            def run_l1():
                with (
                    tc.tile_pool(name="gat1", bufs=3) as gp_,
                    tc.tile_pool(name="idx1", bufs=3) as ip_,
                    tc.tile_pool(name="agg1", bufs=6, space="PSUM") as ap_,
                    tc.tile_pool(name="s1", bufs=24) as sp_,
                    tc.tile_pool(name="post1", bufs=4) as wp_,
                    tc.tile_pool(name="pp1", bufs=2, space="PSUM") as pp_,
                ):
                    mi_by_call = {}
                    for m in L1["mdescs"]:
                        mi_by_call.setdefault(m[0], []).append(m)
                    cur_bi = -1
                    banks = []
                    gpos = {}

                    def agg_ap(g):
                        p = gpos[g]
                        return banks[p // 4][:, (p % 4) * 128:
                                             (p % 4) * 128 + 128]

                    for ci, call in enumerate(L1["calls"]):
                        ntok = call["ntok"]
                        ncols = ntok // 128
                        w = call["w"]
                        bi = call["batch"]
                        if bi != cur_bi:
                            cur_bi = bi
                            groups_b = L1["batches"][bi]
                            gpos = {g: i for i, g in enumerate(groups_b)}
                            banks = [ap_.tile([128, 512], f32, tag="a1",
                                              name="agg1p", space="PSUM")
                                     for _ in range((len(groups_b) + 3) // 4)]
                        gtile = gp_.tile([128, L1["max_ntok"] // 128, FIN],
                                         bf16, tag="g1")
                        idxt = ip_.tile([128, L1["max_ntok"] // 16], i16,
                                        tag="i1")
                        c0 = call["idx_col0"]
                        nc.sync.dma_start(idxt[:, :ntok // 16],
                                          t_idx1[:, c0:c0 + ntok // 16])
                        nc.gpsimd.dma_gather(
                            out_ap=gtile[:, :ncols, :],
                            in_ap=t_xbf[w * WIN:w * WIN + winlen[w], :],
                            idxs_ap=idxt[:, :ntok // 16],
                            num_idxs=ntok,
                            num_idxs_reg=ntok,
                            elem_size=FIN,
                            single_packet=True,
                        )
                        for _, j, g, mi, st, sp in mi_by_call.get(ci, []):
                            S = sp_.tile([128, 128], bf16, tag="S1")
                            nc.vector.tensor_scalar(
                                out=S[:], in0=iota_b[:],
                                scalar1=dst1_t[:, mi:mi + 1],
                                scalar2=coef1_t[:, mi:mi + 1],
                                op0=mybir.AluOpType.is_equal,
                                op1=mybir.AluOpType.mult)
                            # mT[g] += gtile_j^T @ S  -> [fin, dst]
                            nc.tensor.matmul(agg_ap(g), lhsT=gtile[:, j, :],
                                             rhs=S[:], start=mi in l1_first,
                                             stop=False)
                        is_last = (ci + 1 == len(L1["calls"])
                                   or L1["calls"][ci + 1]["batch"] != bi)
                        if is_last:
                            gs_b = L1["batches"][bi]
                            for i, g in enumerate(gs_b):
                                # self loop: mT[g] += xv^T @ diag(dinv^2);
                                # the bank's last matmul carries stop
                                nc.tensor.matmul(
                                    agg_ap(g), lhsT=xown_t[:, g, :],
                                    rhs=diag2_t[:, g, :], start=False,
                                    stop=(i % 4 == 3 or i == len(gs_b) - 1))
                            for g in gs_b:
                                post_l1(g, agg_ap(g), wp_, pp_)
                            g0 = gs_b[0]
                            nc.sync.dma_start(
                                t_g2own[g0 * GSZ:(g0 + len(gs_b)) * GSZ, :]
                                .rearrange("(g p) e -> p g e", p=128),
                                g2own_sb[:, g0:g0 + len(gs_b), :])

            def post_l1(g, mT_p, wp_, pp_):
                mT = wp_.tile([128, 128], bf16, tag="mT")
                nc.scalar.activation(mT[:], mT_p,
                                     mybir.ActivationFunctionType.Copy)
                bank = pp_.tile([128, 512], f32, tag="pb", space="PSUM")
                h1T_p = bank[:FOUT, 0:128]
                g2T_p = bank[:FOUT, 128:256]
                g2p = bank[:, 256:288].bitcast(bf16)   # [128, 64] bf16
                nc.tensor.matmul(h1T_p, lhsT=w1_t[:], rhs=mT[:], start=True,
                                 stop=True)
                z1T = wp_.tile([FOUT, 128], bf16, tag="z1T")
                nc.scalar.activation(z1T[:], h1T_p, actf, bias=b1_t[:, 0:1])
                nc.tensor.matmul(g2T_p, lhsT=w2_t[:], rhs=z1T[:], start=True,
                                 stop=True)
                g2T = wp_.tile([FOUT, 128], bf16, tag="g2T")
                nc.scalar.activation(g2T[:], g2T_p,
                                     mybir.ActivationFunctionType.Copy)
                nc.tensor.transpose(g2p, g2T[:], identb[:FOUT, :FOUT])
                nc.vector.tensor_scalar(
                    out=g2own_sb[:, g, 0:FOUT], in0=g2p,
                    scalar1=dinv_t[:, g:g + 1], scalar2=None,
                    op0=mybir.AluOpType.mult)

            run_l1()

            if PHASE == 1:
                # debug: out = g2own rows (f32)
                with tc.tile_pool(name="dbg", bufs=3) as dp_:
                    for g in range(LGP):
                        td = dp_.tile([128, FOUT], f32, tag="td")
                        nc.vector.tensor_copy(td[:], g2own_sb[:, g, 0:FOUT])
                        nrow = min(GSZ, RS - g * GSZ)
                        nc.sync.dma_start(t_out[g * GSZ:g * GSZ + nrow, :],
                                          td[:nrow, :])
                nc.compile()
                return nc

            # ---------------- layer 2 (src-sharded partials + RS)
            rs_of_batch = {}
            for k, bi in enumerate(pp["rs_batch"]):
                rs_of_batch.setdefault(bi, []).append(k)
            # emit chunk-k finals two batches after its RS fires so their
            # RS-dependent waits never head-of-line-block the DVE queue
            n_bat = len(L2["batches"])
            fin_of_batch = {}
            fin_tail = []
            for k, bi in enumerate(pp["rs_batch"]):
                if bi + 4 < n_bat - 1:
                    fin_of_batch.setdefault(bi + 4, []).append(k)
                else:
                    fin_tail.append(k)

            with (
                tc.tile_pool(name="gat2", bufs=4) as gp_,
                tc.tile_pool(name="idx2", bufs=4) as ip_,
                tc.tile_pool(name="agg2", bufs=8, space="PSUM") as ap_,
                tc.tile_pool(name="s2", bufs=6) as sp_,
                tc.tile_pool(name="pw2", bufs=6) as pw_,
                tc.tile_pool(name="fins", bufs=1) as fs_,
                tc.tile_pool(name="fin", bufs=4) as fp_,
            ):
                OB = 7      # groups per out DMA

                def final_chunk(k):
                    # out = dinv*(z2red[k] + g2own) + b2 for the chunk's
                    # local groups; chunk bounds are group-aligned.
                    ncol = (clen[k] + GSZ - 1) // GSZ
                    st_ = fs_.tile([128, ncol, FOUT], bf16, tag=f"fst{k}",
                                   name=f"fst{k}")
                    full = clen[k] // GSZ
                    if full:
                        nc.sync.dma_start(st_[:, :full, :],
                                          t_z2red[k][:full * GSZ, :]
                                          .rearrange("(s p) e -> p s e",
                                                     p=128))
                    if clen[k] % GSZ:
                        nc.sync.dma_start(
                            st_[:clen[k] % GSZ, full, :],
                            t_z2red[k][full * GSZ:clen[k], :])
                    g0k, g1k = R[k] // GSZ, (R[k + 1] + GSZ - 1) // GSZ
                    ot = None
                    for idx, g in enumerate(range(g0k, g1k)):
                        nrow = min(GSZ, RS - g * GSZ)
                        o = idx % OB
                        if o == 0:
                            ot = fp_.tile([128, OB, FOUT], f32, tag="t3")
                        t1 = fp_.tile([128, FOUT], f32, tag="t1")
                        nc.vector.tensor_tensor(
                            out=t1[:nrow, :], in0=st_[:nrow, idx, :],
                            in1=g2own_sb[:nrow, g, 0:FOUT],
                            op=mybir.AluOpType.add)
                        nc.vector.tensor_scalar(
                            out=t1[:nrow, :], in0=t1[:nrow, :],
                            scalar1=dinv_t[:nrow, g:g + 1], scalar2=None,
                            op0=mybir.AluOpType.mult)
                        nc.vector.tensor_tensor(
                            out=ot[:nrow, o, :], in0=t1[:nrow, :],
                            in1=b2_t[:nrow, :], op=mybir.AluOpType.add)
                        if o == OB - 1 or g == g1k - 1:
                            g0 = g - o
                            if nrow == GSZ:
                                nc.sync.dma_start(
                                    t_out[g0 * GSZ:(g + 1) * GSZ, :]
                                    .rearrange("(g p) e -> p g e", p=128),
                                    ot[:, :o + 1, :])
                            else:
                                if o:
                                    nc.sync.dma_start(
                                        t_out[g0 * GSZ:g * GSZ, :]
                                        .rearrange("(g p) e -> p g e",
                                                   p=128),
                                        ot[:, :o, :])
                                nc.sync.dma_start(
                                    t_out[g * GSZ:g * GSZ + nrow, :],
                                    ot[:nrow, o, :])

                mi_by_call = {}
                for m in L2["mdescs"]:
                    mi_by_call.setdefault(m[0], []).append(m)
                cur_bi = -1
                banks = []
                gpos = {}

                def agg_ap2(g):
                    p = gpos[g]
                    return banks[p // 8][:, (p % 8) * FOUT:
                                         (p % 8) * FOUT + FOUT]

                for ci, call in enumerate(L2["calls"]):
                    ntok = call["ntok"]
                    ncols = ntok // 128
                    bi = call["batch"]
                    if bi != cur_bi:
                        cur_bi = bi
                        groups_b = L2["batches"][bi]
                        gpos = {g: i for i, g in enumerate(groups_b)}
                        banks = [ap_.tile([128, 512], f32, tag="a2",
                                          name="agg2p", space="PSUM")
                                 for _ in range((len(groups_b) + 7) // 8)]
                    gtile = gp_.tile([128, L2["max_ntok"] // 128, FIN],
                                     bf16, tag="g2")
                    idxt = ip_.tile([128, L2["max_ntok"] // 16], i16,
                                    tag="i2")
                    c0 = call["idx_col0"]
                    nc.sync.dma_start(idxt[:, :ntok // 16],
                                      t_idx2[:, c0:c0 + ntok // 16])
                    nc.gpsimd.dma_gather(
                        out_ap=gtile[:, :ncols, :],
                        in_ap=t_g2own[:, :],
                        idxs_ap=idxt[:, :ntok // 16],
                        num_idxs=ntok,
                        num_idxs_reg=ntok,
                        elem_size=FIN,
                        single_packet=True,
                    )
                    for _, j, g, mi, st, sp in mi_by_call.get(ci, []):
                        if mi % 8 == 0:
                            # one DVE op builds S for 8 columns:
                            # s8[p, d, q] = (dst2[p, mi+q] == d)
                            s8 = sp_.tile([128, 128, 8], bf16, tag="S2")
                            nc.vector.tensor_tensor(
                                out=s8[:], in0=iota8_b[:],
                                in1=dst2_t[:, mi:mi + 8].unsqueeze(1)
                                .broadcast_to([128, 128, 8]),
                                op=mybir.AluOpType.is_equal)
                        nc.tensor.matmul(agg_ap2(g), lhsT=s8[:, :, mi % 8],
                                         rhs=gtile[:, j, 0:FOUT],
                                         start=mi in l2_first,
                                         stop=mi in l2_last)
                    is_last = (ci + 1 == len(L2["calls"])
                               or L2["calls"][ci + 1]["batch"] != bi)
                    if is_last:
                        gs_b = L2["batches"][bi]
                        stages = []
                        for b in range(len(banks)):
                            nct = min(8, len(gs_b) - b * 8)
                            stg = pw_.tile([128, 8, FOUT], bf16, tag="ps")
                            nc.scalar.activation(
                                stg[:, :nct, :],
                                banks[b][:, :nct * FOUT].rearrange(
                                    "p (c e) -> p c e", e=FOUT),
                                mybir.ActivationFunctionType.Copy)
                            stages.append(stg)
                        for op in _l2_batch_dmas(gs_b, R):
                            if op[0] == "full":
                                _, k, dr, b, c0, c1 = op
                                nc.sync.dma_start(
                                    t_part[k][dr:dr + (c1 - c0) * 128, :]
                                    .rearrange("(c p) e -> p c e", p=128),
                                    stages[b][:, c0:c1, :])
                            else:
                                _, k, dr, b, c, p0, p1 = op
                                nc.sync.dma_start(
                                    t_part[k][dr:dr + (p1 - p0), :],
                                    stages[b][p0:p1, c, :])
                        for k in rs_of_batch.get(bi, []):
                            nc.gpsimd.collective_compute(
                                "ReduceScatter", mybir.AluOpType.add,
                                replica_groups=[list(range(NC))],
                                ins=[t_part[k][:, :]],
                                outs=[t_z2red[k][:, :]])
                        for k in fin_of_batch.get(bi, []):
                            final_chunk(k)
                for k in fin_tail:
                    final_chunk(k)

    nc.compile()
    return nc


def _in_maps(pp):
    maps = []
    for c in range(NC):
        maps.append({
            "xbf": pp["xbf"],
            "idx1": pp["L1"]["idx"][c],
            "idx2": pp["L2"]["idx"][c],
            "dst1": pp["L1"]["dst"][c],
            "coef1": pp["L1"]["coef"][c],
            "dst2": pp["L2"]["dst"][c].astype(BF16),
            "dinv_pc": pp["dinv_pc"][c],
            "diag2": pp["diag2_pc"][c],
            "xown": pp["xown_pc"][c],
            "W1bf": pp["W1bf"], "W2bf": pp["W2bf"],
            "b1col": pp["b1col"], "b2bc": pp["b2bc"],
        })
    return maps


def _run(inputs, act="gelu", trace=False, use_sim=False, trace_kwargs=None):
    x = np.ascontiguousarray(np.asarray(inputs["x"], np.float32))
    key = (hash(np.asarray(inputs["edge_index"]).tobytes()), act, PHASE)
    if key not in _cache:
        pp = _preprocess(x, np.asarray(inputs["edge_index"]),
                         inputs["W1"], inputs["b1"], inputs["W2"],
                         inputs["b2"])
        nc = _build_nc(pp, act=act)
        _cache.clear()
        _cache[key] = (pp, nc)
    pp, nc = _cache[key]

    in_maps = _in_maps(pp)
    if use_sim:
        from concourse.bass_interp import MultiCoreSim
        sim = MultiCoreSim(nc, num_cores=NC)
        for ci, core in sim.cores.items():
            for k, v in in_maps[ci].items():
                core.tensor(k)[:] = v
        sim.simulate()
        outs = [np.array(core.tensor("out"))
                for _, core in sorted(sim.cores.items())]
        return np.concatenate(outs, 0)[pp["slot_of"]], None
    from concourse.bass_utils import run_bass_kernel_spmd
    res = run_bass_kernel_spmd(nc, in_maps, core_ids=list(range(NC)),
                               trace=trace, **(trace_kwargs or {}))
    out = np.concatenate([res.results[c]["out"] for c in range(NC)], 0)
    return out[pp["slot_of"]], res


def kernel(**inputs) -> np.ndarray:
    out, _ = _run(inputs)
    return out


def bench(inputs, act="gelu", iters=8):
    """Measure per-execution device time by chaining `iters` executions of
    the NEFF inside one jit and comparing against a 1-iteration call."""
    import time
    import jax
    from jax.sharding import Mesh, PartitionSpec
    from jax.experimental.shard_map import shard_map
    from concourse import bass2jax as b2j

    key = (hash(np.asarray(inputs["edge_index"]).tobytes()), act, PHASE)
    if key not in _cache:
        _run(inputs, act=act)   # build + correctness path
    pp, nc = _cache[key]
    b2j.install_neuronx_cc_hook()

    in_maps = _in_maps(pp)

    in_names, out_names, out_avals, zero_outs = [], [], [], []
    import concourse.mybir as mb
    pid_name = (nc.partition_id_tensor.name
                if nc.partition_id_tensor is not None else None)
    for alloc in nc.m.functions[0].allocations:
        if not isinstance(alloc, mb.MemoryLocationSet):
            continue
        name = alloc.memorylocations[0].name
        if alloc.kind == "ExternalInput":
            if name == pid_name:
                continue
            in_names.append(name)
        elif alloc.kind == "ExternalOutput":
            out_names.append(name)
            shape = tuple(alloc.tensor_shape)
            dtype = mb.dt.np(alloc.dtype)
            out_avals.append(jax.core.ShapedArray(shape, dtype))
            zero_outs.append(np.zeros(shape, dtype))
    n_params = len(in_names)
    all_names = in_names + out_names
    if pid_name is not None:
        all_names = all_names + [pid_name]

    def one_call(params, outs_in):
        extra = ([b2j.partition_id_tensor()] if pid_name is not None else [])
        outs = b2j._bass_exec_p.bind(
            *params, *outs_in, *extra,
            out_avals=tuple(out_avals),
            in_names=tuple(all_names),
            out_names=tuple(out_names),
            lowering_input_output_aliases=(),
            sim_require_finite=True,
            sim_require_nnan=True,
            nc=nc,
        )
        return list(outs)

    def _body(*args):
        params = list(args[:n_params])
        outs = list(args[n_params:])
        outs = one_call(params, outs)
        return tuple(outs)

    devices = jax.devices()[:NC]
    mesh = Mesh(np.asarray(devices), ("core",))
    specs = (PartitionSpec("core"),)
    per_core = [[np.asarray(m[nm]) for nm in in_names] for m in in_maps]
    concat_in = [np.concatenate([per_core[c][i] for c in range(NC)], 0)
                 for i in range(n_params)]
    concat_zeros = [np.zeros((NC * z.shape[0], *z.shape[1:]), z.dtype)
                    for z in zero_outs]

    nin = n_params + len(out_names)
    fn = jax.jit(shard_map(_body, mesh=mesh,
                           in_specs=specs * nin,
                           out_specs=specs * len(out_names),
                           check_rep=False),
                 donate_argnums=tuple(range(n_params, nin)))
    from jax.sharding import NamedSharding
    shard = NamedSharding(mesh, PartitionSpec("core"))
    dev_in = [jax.device_put(a, shard) for a in concat_in]
    outs = [jax.device_put(a, shard) for a in concat_zeros]
    outs = fn(*dev_in, *outs)          # warm: compile + first exec
    jax.block_until_ready(outs)

    results = {}
    for k in (1, iters):
        best = None
        for _ in range(3):
            t0 = time.perf_counter()
            o = outs
            for _ in range(k):
                o = fn(*dev_in, *o)
            jax.block_until_ready(o)
            dt = time.perf_counter() - t0
            outs = o
            best = dt if best is None else min(best, dt)
        results[k] = best
    per_iter_ns = (results[iters] - results[1]) / (iters - 1) * 1e9
    return per_iter_ns, results
